# revision 31
# baseline (speedup 1.0000x reference)
"""IndRNN kernel for 8 Trainium2 NeuronCores.

Math: h_t = relu(x_t @ W + b + u * h_{t-1}), h_0 = ones.  Output all h_t.

Strategy
--------
- Data-parallel over batch: B=32 -> 4 batches per core.
- Host prep: pre-transpose x to [B, D, T] (contraction dim on partitions)
  and cast to fp16; W to fp16.  fp16 keeps quantization ~2.4e-4 while
  doubling DMA and PE throughput vs fp32.
- Device per core: xw = W^T @ x^T produced as [h, t] tiles in PSUM; ACT
  copies to SBUF with scale=-1 and bias=-b, yielding d_t = -(xw_t + b).
- Recurrence via two exact hardware scans (DVE tensor_tensor_scan):
      beta_t = u * beta_{t-1} + d_t        (op0=mult, op1=add,  init 0)
      v_t    = max(u * v_{t-1}, beta_t)    (op0=mult, op1=max,  init 1)
  Identity: h_t = v_t - beta_t.  Proof: if v_t = h_t + beta_t then
      max(u v_{t-1}, beta_t) = max(u h_{t-1} + u beta_{t-1}, beta_t)
                             = max(u h_{t-1} - d_t + beta_t, beta_t)
                             = relu(u h_{t-1} + xw_t + b) + beta_t.
  No overflow: |beta| <= max|xw| / (1 - u) which is small here; no
  chunk chaining, no lanes splitting, no rescale tables.
- Both scans emit fp16 (the scan state stays fp32 internally); h = v - beta
  runs on DVE in all-fp16 2x mode (0.5 cy/col).  GpSimd is avoided
  entirely (measured ~0.2 efficiency + 1.6us semaphore costs, and its SBUF
  traffic slows the DVE scans).  The first batch's scans are chunked (1024)
  and chained through the previous chunk's last column so the first scan
  starts after two matmul chunks instead of eight (hides the d-fill ramp);
  later batches use single full-length scans (lower fixed overhead).
  PSUM pool depth 4 keeps the matmul stream well ahead of the ACT copies
  (intermittent tail-chunk corruption observed at depth 2).
- Output DMA'd as [b, g, 128, T] fp16; host reassembles to [B, T, H] fp32.
"""

import sys

for _p in ("/opt/trn_rl_repo",):
    if _p not in sys.path:
        sys.path.insert(0, _p)

from contextlib import ExitStack

import numpy as np

import concourse.bass as bass
import concourse.tile as tile
from concourse import bacc, mybir
from concourse.bass_utils import run_bass_kernel_spmd

F32 = mybir.dt.float32
F16 = mybir.dt.float16
BF16 = mybir.dt.bfloat16
ALU = mybir.AluOpType
ACTF = mybir.ActivationFunctionType

B, T, D, H = 32, 4096, 256, 256
NCORES = 8
BLOC = B // NCORES  # batches per core
NT = 512            # matmul / psum chunk along time
NCHUNK = T // NT
LSC = 1024         # scan chunk length (chained)


def _build(nc):
    xt_d = nc.declare_dram_parameter("xt", [BLOC, D, T], F16, isOutput=False)
    w_d = nc.declare_dram_parameter("w", [D, H], F16, isOutput=False)
    bneg_d = nc.declare_dram_parameter("bneg", [H, 1], F32, isOutput=False)
    uc_d = nc.declare_dram_parameter("ucol", [H, 1], F32, isOutput=False)
    out_d = nc.declare_dram_parameter("out", [BLOC, 2, 128, T], F16, isOutput=True)

    with tile.TileContext(nc) as tc, ExitStack() as ctx:
        const = ctx.enter_context(tc.tile_pool(name="const", bufs=1))
        xt_pool = ctx.enter_context(tc.tile_pool(name="xt", bufs=3))
        psum_pool = ctx.enter_context(
            tc.tile_pool(name="psum", bufs=4, space=bass.MemorySpace.PSUM)
        )
        d_pool = ctx.enter_context(tc.tile_pool(name="d", bufs=2))
        beta_pool = ctx.enter_context(tc.tile_pool(name="beta", bufs=2))
        v_pool = ctx.enter_context(tc.tile_pool(name="v", bufs=2))
        h_pool = ctx.enter_context(tc.tile_pool(name="h", bufs=2))

        w_sb = []
        for dh in range(2):
            wt = const.tile([128, H], F16, tag=f"w{dh}")
            nc.sync.dma_start(wt[:, :], w_d[dh * 128 : (dh + 1) * 128, :])
            w_sb.append(wt)
        u_sb, bn_sb = [], []
        for g in range(2):
            ut = const.tile([128, 1], F32, tag=f"u{g}")
            nc.sync.dma_start(ut[:, :], uc_d[g * 128 : (g + 1) * 128, :])
            u_sb.append(ut)
            bt = const.tile([128, 1], F32, tag=f"bn{g}")
            nc.sync.dma_start(bt[:, :], bneg_d[g * 128 : (g + 1) * 128, :])
            bn_sb.append(bt)

        for b in range(BLOC):
            d_t0 = d_pool.tile([128, T], F32, tag="d0")
            d_t1 = d_pool.tile([128, T], F32, tag="d1")
            d_t = [d_t0, d_t1]
            for c2 in range(NCHUNK // 2):
                xts = []
                for dh in range(2):
                    xtt = xt_pool.tile([128, 2 * NT], F16, tag=f"x{dh}")
                    nc.sync.dma_start(
                        xtt[:, :],
                        xt_d[
                            b,
                            dh * 128 : (dh + 1) * 128,
                            c2 * 2 * NT : (c2 + 1) * 2 * NT,
                        ],
                    )
                    xts.append(xtt)
                for ci in range(2):
                    c = 2 * c2 + ci
                    for g in range(2):
                        ps = psum_pool.tile([128, NT], F32, tag=f"p{g}")
                        for dh in range(2):
                            nc.tensor.matmul(
                                ps[:, :],
                                w_sb[dh][:, g * 128 : (g + 1) * 128],
                                xts[dh][:, ci * NT : (ci + 1) * NT],
                                start=(dh == 0),
                                stop=(dh == 1),
                            )
                        nc.scalar.activation(
                            d_t[g][:, c * NT : (c + 1) * NT],
                            ps[:, :],
                            ACTF.Identity,
                            bias=bn_sb[g][:, :],
                            scale=-1.0,
                        )
            for g in range(2):
                bt = beta_pool.tile([128, T], F16, tag=f"b{g}")
                vt = v_pool.tile([128, T], F16, tag=f"v{g}")
                # b=0: chunked scans (chained via the previous chunk's last
                # column) so the first scan starts after 2 matmul chunks,
                # not 8 -- hides the d-tile fill latency.  Later batches
                # use single full-length scans (lower fixed overhead).
                last = b == BLOC - 1
                lsc = LSC if (b == 0 or last) else T
                ht = h_pool.tile([128, T], F16, tag=f"h{g}")
                for s0 in range(0, T, lsc):
                    ub = u_sb[g][:, 0:1].broadcast_to([128, lsc])
                    bi = 0.0 if s0 == 0 else bt[:, s0 - 1 : s0]
                    nc.vector.tensor_tensor_scan(
                        bt[:, s0 : s0 + lsc],
                        ub,
                        d_t[g][:, s0 : s0 + lsc],
                        bi,
                        op0=ALU.mult,
                        op1=ALU.add,
                    )
                    vi = 1.0 if s0 == 0 else vt[:, s0 - 1 : s0]
                    nc.vector.tensor_tensor_scan(
                        vt[:, s0 : s0 + lsc],
                        ub,
                        bt[:, s0 : s0 + lsc],
                        vi,
                        op0=ALU.mult,
                        op1=ALU.max,
                    )
                    if last:
                        # interleave the subtract so only one small chunk
                        # remains after the final scan (shorter tail)
                        nc.vector.tensor_tensor(
                            ht[:, s0 : s0 + lsc],
                            vt[:, s0 : s0 + lsc],
                            bt[:, s0 : s0 + lsc],
                            op=ALU.subtract,
                        )
                if not last:
                    # h = v - beta, all-fp16 (2x DVE mode)
                    for s0 in range(0, T, T // 2):
                        nc.vector.tensor_tensor(
                            ht[:, s0 : s0 + T // 2],
                            vt[:, s0 : s0 + T // 2],
                            bt[:, s0 : s0 + T // 2],
                            op=ALU.subtract,
                        )
                nc.sync.dma_start(out_d[b, g], ht[:, :])


def _host_prep(x, W, b, u):
    x = np.asarray(x, np.float32)
    W = np.asarray(W, np.float32)
    b = np.asarray(b, np.float32)
    u = np.asarray(u, np.float32)

    xt = np.ascontiguousarray(np.swapaxes(x, 1, 2).astype(np.float16))  # [B,D,T]
    common = {
        "w": np.ascontiguousarray(W.astype(np.float16)),
        "bneg": np.ascontiguousarray((-b)[:, None].astype(np.float32)),
        "ucol": np.ascontiguousarray(u[:, None].astype(np.float32)),
    }
    in_maps = []
    for c in range(NCORES):
        m = dict(common)
        m["xt"] = np.ascontiguousarray(xt[c * BLOC : (c + 1) * BLOC])
        in_maps.append(m)
    return in_maps


# set by test harnesses to profile: kernel() stores the raw results here
LAST_RESULT = None


def kernel(x, W, b, u):
    global LAST_RESULT
    import os

    in_maps = _host_prep(x, W, b, u)

    nc = bacc.Bacc("TRN2", target_bir_lowering=False, debug=False)
    _build(nc)
    nc.compile()

    trace = bool(os.environ.get("INDRNN_TRACE"))
    res = run_bass_kernel_spmd(
        nc, in_maps, core_ids=list(range(NCORES)), trace=trace
    )
    LAST_RESULT = res
    out_dev = np.concatenate([r["out"] for r in res.results], axis=0)  # [B,2,128,T]

    out = out_dev.reshape(B, H, T).astype(np.float32)
    return np.ascontiguousarray(np.swapaxes(out, 1, 2))


# revision 32
# speedup vs baseline: 1.1760x; 1.1760x over previous
"""IndRNN kernel for 8 Trainium2 NeuronCores.

Math: h_t = relu(x_t @ W + b + u * h_{t-1}), h_0 = ones.  Output all h_t.

Strategy
--------
- Data-parallel over batch: B=32 -> 4 batches per core.
- Host prep: pre-transpose x to [B, D, T] (contraction dim on partitions)
  and cast to fp16; W to fp16.  fp16 keeps quantization ~2.4e-4 while
  doubling DMA and PE throughput vs fp32.
- Device per core: xw = W^T @ x^T produced as [h, t] tiles in PSUM; ACT
  copies to SBUF with scale=-1 and bias=-b, yielding d_t = -(xw_t + b).
- Recurrence via two exact hardware scans (DVE tensor_tensor_scan):
      beta_t = u * beta_{t-1} + d_t        (op0=mult, op1=add,  init 0)
      v_t    = max(u * v_{t-1}, beta_t)    (op0=mult, op1=max,  init 1)
  Identity: h_t = v_t - beta_t.  Proof: if v_t = h_t + beta_t then
      max(u v_{t-1}, beta_t) = max(u h_{t-1} + u beta_{t-1}, beta_t)
                             = max(u h_{t-1} - d_t + beta_t, beta_t)
                             = relu(u h_{t-1} + xw_t + b) + beta_t.
  No overflow: |beta| <= max|xw| / (1 - u) which is small here; no
  chunk chaining, no lanes splitting, no rescale tables.
- Both scans emit fp16 (the scan state stays fp32 internally); h = v - beta
  runs on DVE in all-fp16 2x mode (0.5 cy/col).  GpSimd is avoided
  entirely (measured ~0.2 efficiency + 1.6us semaphore costs, and its SBUF
  traffic slows the DVE scans).  The first batch's scans are chunked (1024)
  and chained through the previous chunk's last column so the first scan
  starts after two matmul chunks instead of eight (hides the d-fill ramp);
  later batches use single full-length scans (lower fixed overhead).
  PSUM pool depth 4 keeps the matmul stream well ahead of the ACT copies
  (intermittent tail-chunk corruption observed at depth 2).
- Output DMA'd as [b, g, 128, T] fp16; host reassembles to [B, T, H] fp32.
"""

import sys

for _p in ("/opt/trn_rl_repo",):
    if _p not in sys.path:
        sys.path.insert(0, _p)

from contextlib import ExitStack

import numpy as np

import concourse.bass as bass
import concourse.tile as tile
from concourse import bacc, mybir
from concourse.bass_utils import run_bass_kernel_spmd

F32 = mybir.dt.float32
F16 = mybir.dt.float16
BF16 = mybir.dt.bfloat16
ALU = mybir.AluOpType
ACTF = mybir.ActivationFunctionType

B, T, D, H = 32, 4096, 256, 256
NCORES = 8
BLOC = B // NCORES  # batches per core
NT = 512            # matmul / psum chunk along time
NCHUNK = T // NT
LSC = 1024         # scan chunk length (chained)


def _build(nc):
    xt_d = nc.declare_dram_parameter("xt", [BLOC, D, T], F16, isOutput=False)
    w_d = nc.declare_dram_parameter("w", [D, H], F16, isOutput=False)
    bneg_d = nc.declare_dram_parameter("bneg", [H, 1], F32, isOutput=False)
    uc_d = nc.declare_dram_parameter("ucol", [H, 1], F32, isOutput=False)
    out_d = nc.declare_dram_parameter("out", [BLOC, 2, 128, T], F16, isOutput=True)

    with tile.TileContext(nc) as tc, ExitStack() as ctx:
        const = ctx.enter_context(tc.tile_pool(name="const", bufs=1))
        xt_pool = ctx.enter_context(tc.tile_pool(name="xt", bufs=3))
        psum_pool = ctx.enter_context(
            tc.tile_pool(name="psum", bufs=4, space=bass.MemorySpace.PSUM)
        )
        d_pool = ctx.enter_context(tc.tile_pool(name="d", bufs=2))
        beta_pool = ctx.enter_context(tc.tile_pool(name="beta", bufs=2))
        v_pool = ctx.enter_context(tc.tile_pool(name="v", bufs=2))
        h_pool = ctx.enter_context(tc.tile_pool(name="h", bufs=2))

        w_sb = []
        for dh in range(2):
            wt = const.tile([128, H], F16, tag=f"w{dh}")
            nc.sync.dma_start(wt[:, :], w_d[dh * 128 : (dh + 1) * 128, :])
            w_sb.append(wt)
        u_sb, bn_sb = [], []
        for g in range(2):
            ut = const.tile([128, 1], F32, tag=f"u{g}")
            nc.sync.dma_start(ut[:, :], uc_d[g * 128 : (g + 1) * 128, :])
            u_sb.append(ut)
            bt = const.tile([128, 1], F32, tag=f"bn{g}")
            nc.sync.dma_start(bt[:, :], bneg_d[g * 128 : (g + 1) * 128, :])
            bn_sb.append(bt)

        for b in range(BLOC):
            d_t0 = d_pool.tile([128, T], F32, tag="d0")
            d_t1 = d_pool.tile([128, T], F32, tag="d1")
            d_t = [d_t0, d_t1]
            for c in range(NCHUNK):
                xts = []
                for dh in range(2):
                    xtt = xt_pool.tile([128, NT], F16, tag=f"x{dh}")
                    nc.sync.dma_start(
                        xtt[:, :],
                        xt_d[b, dh * 128 : (dh + 1) * 128, c * NT : (c + 1) * NT],
                    )
                    xts.append(xtt)
                for g in range(2):
                    ps = psum_pool.tile([128, NT], F32, tag=f"p{g}")
                    for dh in range(2):
                        nc.tensor.matmul(
                            ps[:, :],
                            w_sb[dh][:, g * 128 : (g + 1) * 128],
                            xts[dh][:, :],
                            start=(dh == 0),
                            stop=(dh == 1),
                        )
                    nc.scalar.activation(
                        d_t[g][:, c * NT : (c + 1) * NT],
                        ps[:, :],
                        ACTF.Identity,
                        bias=bn_sb[g][:, :],
                        scale=-1.0,
                    )
            for g in range(2):
                bt = beta_pool.tile([128, T], F16, tag=f"b{g}")
                vt = v_pool.tile([128, T], F16, tag=f"v{g}")
                # b=0: chunked scans (chained via the previous chunk's last
                # column) so the first scan starts after 2 matmul chunks,
                # not 8 -- hides the d-tile fill latency.  Later batches
                # use single full-length scans (lower fixed overhead).
                last = b == BLOC - 1
                lsc = LSC if (b == 0 or last) else T
                ht = h_pool.tile([128, T], F16, tag=f"h{g}")
                for s0 in range(0, T, lsc):
                    ub = u_sb[g][:, 0:1].broadcast_to([128, lsc])
                    bi = 0.0 if s0 == 0 else bt[:, s0 - 1 : s0]
                    nc.vector.tensor_tensor_scan(
                        bt[:, s0 : s0 + lsc],
                        ub,
                        d_t[g][:, s0 : s0 + lsc],
                        bi,
                        op0=ALU.mult,
                        op1=ALU.add,
                    )
                    vi = 1.0 if s0 == 0 else vt[:, s0 - 1 : s0]
                    nc.vector.tensor_tensor_scan(
                        vt[:, s0 : s0 + lsc],
                        ub,
                        bt[:, s0 : s0 + lsc],
                        vi,
                        op0=ALU.mult,
                        op1=ALU.max,
                    )
                    if last:
                        # interleave the subtract so only one small chunk
                        # remains after the final scan (shorter tail)
                        nc.vector.tensor_tensor(
                            ht[:, s0 : s0 + lsc],
                            vt[:, s0 : s0 + lsc],
                            bt[:, s0 : s0 + lsc],
                            op=ALU.subtract,
                        )
                if not last:
                    # h = v - beta, all-fp16 (2x DVE mode)
                    for s0 in range(0, T, T // 2):
                        nc.vector.tensor_tensor(
                            ht[:, s0 : s0 + T // 2],
                            vt[:, s0 : s0 + T // 2],
                            bt[:, s0 : s0 + T // 2],
                            op=ALU.subtract,
                        )
                nc.sync.dma_start(out_d[b, g], ht[:, :])


def _host_prep(x, W, b, u):
    x = np.asarray(x, np.float32)
    W = np.asarray(W, np.float32)
    b = np.asarray(b, np.float32)
    u = np.asarray(u, np.float32)

    xt = np.ascontiguousarray(np.swapaxes(x, 1, 2).astype(np.float16))  # [B,D,T]
    common = {
        "w": np.ascontiguousarray(W.astype(np.float16)),
        "bneg": np.ascontiguousarray((-b)[:, None].astype(np.float32)),
        "ucol": np.ascontiguousarray(u[:, None].astype(np.float32)),
    }
    in_maps = []
    for c in range(NCORES):
        m = dict(common)
        m["xt"] = np.ascontiguousarray(xt[c * BLOC : (c + 1) * BLOC])
        in_maps.append(m)
    return in_maps


# set by test harnesses to profile: kernel() stores the raw results here
LAST_RESULT = None


def kernel(x, W, b, u):
    global LAST_RESULT
    import os

    in_maps = _host_prep(x, W, b, u)

    nc = bacc.Bacc("TRN2", target_bir_lowering=False, debug=False)
    _build(nc)
    nc.compile()

    trace = bool(os.environ.get("INDRNN_TRACE"))
    res = run_bass_kernel_spmd(
        nc, in_maps, core_ids=list(range(NCORES)), trace=trace
    )
    LAST_RESULT = res
    out_dev = np.concatenate([r["out"] for r in res.results], axis=0)  # [B,2,128,T]

    out = out_dev.reshape(B, H, T).astype(np.float32)
    return np.ascontiguousarray(np.swapaxes(out, 1, 2))


# revision 33
# speedup vs baseline: 1.2058x; 1.0254x over previous
"""IndRNN kernel for 8 Trainium2 NeuronCores.

Math: h_t = relu(x_t @ W + b + u * h_{t-1}), h_0 = ones.  Output all h_t.

Strategy
--------
- Data-parallel over batch: B=32 -> 4 batches per core.
- Host prep: pre-transpose x to [B, D, T] (contraction dim on partitions)
  and cast to fp16; W to fp16.  fp16 keeps quantization ~2.4e-4 while
  doubling DMA and PE throughput vs fp32.
- Device per core: xw = W^T @ x^T produced as [h, t] tiles in PSUM; ACT
  copies to SBUF with scale=-1 and bias=-b, yielding d_t = -(xw_t + b).
- Recurrence via two exact hardware scans (DVE tensor_tensor_scan):
      beta_t = u * beta_{t-1} + d_t        (op0=mult, op1=add,  init 0)
      v_t    = max(u * v_{t-1}, beta_t)    (op0=mult, op1=max,  init 1)
  Identity: h_t = v_t - beta_t.  Proof: if v_t = h_t + beta_t then
      max(u v_{t-1}, beta_t) = max(u h_{t-1} + u beta_{t-1}, beta_t)
                             = max(u h_{t-1} - d_t + beta_t, beta_t)
                             = relu(u h_{t-1} + xw_t + b) + beta_t.
  No overflow: |beta| <= max|xw| / (1 - u) which is small here; no
  chunk chaining, no lanes splitting, no rescale tables.
- Both scans emit fp16 (the scan state stays fp32 internally); h = v - beta
  runs on DVE in all-fp16 2x mode (0.5 cy/col).  GpSimd is avoided
  entirely (measured ~0.2 efficiency + 1.6us semaphore costs, and its SBUF
  traffic slows the DVE scans).  The first batch's scans are chunked (1024)
  and chained through the previous chunk's last column so the first scan
  starts after two matmul chunks instead of eight (hides the d-fill ramp);
  later batches use single full-length scans (lower fixed overhead).
  PSUM pool depth 4 keeps the matmul stream well ahead of the ACT copies
  (intermittent tail-chunk corruption observed at depth 2).
- Output DMA'd as [b, g, 128, T] fp16; host reassembles to [B, T, H] fp32.
"""

import sys

for _p in ("/opt/trn_rl_repo",):
    if _p not in sys.path:
        sys.path.insert(0, _p)

from contextlib import ExitStack

import numpy as np

import concourse.bass as bass
import concourse.tile as tile
from concourse import bacc, mybir
from concourse.bass_utils import run_bass_kernel_spmd

F32 = mybir.dt.float32
F16 = mybir.dt.float16
BF16 = mybir.dt.bfloat16
ALU = mybir.AluOpType
ACTF = mybir.ActivationFunctionType

B, T, D, H = 32, 4096, 256, 256
NCORES = 8
BLOC = B // NCORES  # batches per core
NT = 512            # matmul / psum chunk along time
NCHUNK = T // NT
LSC = 1024         # scan chunk length (chained)


def _build(nc):
    xt_d = nc.declare_dram_parameter("xt", [BLOC, D, T], F16, isOutput=False)
    w_d = nc.declare_dram_parameter("w", [D, H], F16, isOutput=False)
    bneg_d = nc.declare_dram_parameter("bneg", [H, 1], F32, isOutput=False)
    uc_d = nc.declare_dram_parameter("ucol", [H, 1], F32, isOutput=False)
    out_d = nc.declare_dram_parameter("out", [BLOC, 2, 128, T], F16, isOutput=True)

    with tile.TileContext(nc) as tc, ExitStack() as ctx:
        const = ctx.enter_context(tc.tile_pool(name="const", bufs=1))
        xt_pool = ctx.enter_context(tc.tile_pool(name="xt", bufs=3))
        psum_pool = ctx.enter_context(
            tc.tile_pool(name="psum", bufs=4, space=bass.MemorySpace.PSUM)
        )
        d_pool = ctx.enter_context(tc.tile_pool(name="d", bufs=2))
        beta_pool = ctx.enter_context(tc.tile_pool(name="beta", bufs=2))
        v_pool = ctx.enter_context(tc.tile_pool(name="v", bufs=2))
        h_pool = ctx.enter_context(tc.tile_pool(name="h", bufs=2))

        w_sb = []
        for dh in range(2):
            wt = const.tile([128, H], F16, tag=f"w{dh}")
            nc.sync.dma_start(wt[:, :], w_d[dh * 128 : (dh + 1) * 128, :])
            w_sb.append(wt)
        u_sb, bn_sb = [], []
        for g in range(2):
            ut = const.tile([128, 1], F32, tag=f"u{g}")
            nc.sync.dma_start(ut[:, :], uc_d[g * 128 : (g + 1) * 128, :])
            u_sb.append(ut)
            bt = const.tile([128, 1], F32, tag=f"bn{g}")
            nc.sync.dma_start(bt[:, :], bneg_d[g * 128 : (g + 1) * 128, :])
            bn_sb.append(bt)

        for b in range(BLOC):
            d_t0 = d_pool.tile([128, T], F32, tag="d0")
            d_t1 = d_pool.tile([128, T], F32, tag="d1")
            d_t = [d_t0, d_t1]
            for c in range(NCHUNK):
                xts = []
                for dh in range(2):
                    xtt = xt_pool.tile([128, NT], F16, tag=f"x{dh}")
                    nc.sync.dma_start(
                        xtt[:, :],
                        xt_d[b, dh * 128 : (dh + 1) * 128, c * NT : (c + 1) * NT],
                    )
                    xts.append(xtt)
                for g in range(2):
                    ps = psum_pool.tile([128, NT], F32, tag=f"p{g}")
                    for dh in range(2):
                        nc.tensor.matmul(
                            ps[:, :],
                            w_sb[dh][:, g * 128 : (g + 1) * 128],
                            xts[dh][:, :],
                            start=(dh == 0),
                            stop=(dh == 1),
                        )
                    nc.scalar.activation(
                        d_t[g][:, c * NT : (c + 1) * NT],
                        ps[:, :],
                        ACTF.Identity,
                        bias=bn_sb[g][:, :],
                        scale=-1.0,
                    )
            for g in range(2):
                bt = beta_pool.tile([128, T], F16, tag=f"b{g}")
                vt = v_pool.tile([128, T], F16, tag=f"v{g}")
                # b=0: chunked scans (chained via the previous chunk's last
                # column) so the first scan starts after 2 matmul chunks,
                # not 8 -- hides the d-tile fill latency.  Later batches
                # use single full-length scans (lower fixed overhead).
                lsc = LSC if b == 0 else T
                for s0 in range(0, T, lsc):
                    ub = u_sb[g][:, 0:1].broadcast_to([128, lsc])
                    bi = 0.0 if s0 == 0 else bt[:, s0 - 1 : s0]
                    nc.vector.tensor_tensor_scan(
                        bt[:, s0 : s0 + lsc],
                        ub,
                        d_t[g][:, s0 : s0 + lsc],
                        bi,
                        op0=ALU.mult,
                        op1=ALU.add,
                    )
                    vi = 1.0 if s0 == 0 else vt[:, s0 - 1 : s0]
                    nc.vector.tensor_tensor_scan(
                        vt[:, s0 : s0 + lsc],
                        ub,
                        bt[:, s0 : s0 + lsc],
                        vi,
                        op0=ALU.mult,
                        op1=ALU.max,
                    )
                # h = v - beta, all-fp16 (2x DVE mode), chunked to overlap
                # the DMA writeback with the next scans
                ht = h_pool.tile([128, T], F16, tag=f"h{g}")
                for s0 in range(0, T, T // 2):
                    nc.vector.tensor_tensor(
                        ht[:, s0 : s0 + T // 2],
                        vt[:, s0 : s0 + T // 2],
                        bt[:, s0 : s0 + T // 2],
                        op=ALU.subtract,
                    )
                nc.sync.dma_start(out_d[b, g], ht[:, :])


def _host_prep(x, W, b, u):
    x = np.asarray(x, np.float32)
    W = np.asarray(W, np.float32)
    b = np.asarray(b, np.float32)
    u = np.asarray(u, np.float32)

    xt = np.ascontiguousarray(np.swapaxes(x, 1, 2).astype(np.float16))  # [B,D,T]
    common = {
        "w": np.ascontiguousarray(W.astype(np.float16)),
        "bneg": np.ascontiguousarray((-b)[:, None].astype(np.float32)),
        "ucol": np.ascontiguousarray(u[:, None].astype(np.float32)),
    }
    in_maps = []
    for c in range(NCORES):
        m = dict(common)
        m["xt"] = np.ascontiguousarray(xt[c * BLOC : (c + 1) * BLOC])
        in_maps.append(m)
    return in_maps


# set by test harnesses to profile: kernel() stores the raw results here
LAST_RESULT = None


def kernel(x, W, b, u):
    global LAST_RESULT
    import os

    in_maps = _host_prep(x, W, b, u)

    nc = bacc.Bacc("TRN2", target_bir_lowering=False, debug=False)
    _build(nc)
    nc.compile()

    trace = bool(os.environ.get("INDRNN_TRACE"))
    res = run_bass_kernel_spmd(
        nc, in_maps, core_ids=list(range(NCORES)), trace=trace
    )
    LAST_RESULT = res
    out_dev = np.concatenate([r["out"] for r in res.results], axis=0)  # [B,2,128,T]

    out = out_dev.reshape(B, H, T).astype(np.float32)
    return np.ascontiguousarray(np.swapaxes(out, 1, 2))
